# revision 21
# baseline (speedup 1.0000x reference)
"""CP-decomposed 3D conv (AirConv3D) on 8 Trainium2 NeuronCores.

Math (reference):
  out[o,X,Y,Z] = sum_{i,j,l,c,r} xp[c,X+i,Y+j,Z+l] * U_kh[i,r]*U_kw[j,r]*U_kd[l,r]
                  * U_cin[c,r]*U_cout[r,o] + bias[o]
  xp = zero-padded x (pad 1), kernel 3x3x3, CP rank 53.

Device pipeline per core (H-sharded, 7 output rows + 1-row halo each side):
  S1  TensorE: K=(i,c)=96 matmul folding the H-tap conv + channel contraction
      -> t2[r, Y58, Z30] per z-half (PSUM partition blocks 0/64 via col groups)
  E1  ACT: evict PSUM -> bf16 SBUF (T2)
  YC  DVE: 3-tap y-conv with per-partition scalars (3x tensor_scalar + 2x TT add)
  S5  TensorE: 3 accumulated z-shifted matmuls folding the D-tap conv and
      rank->cout contraction (K=53 per z-half; zh blocks run as concurrent
      row-group tiles)
  E2  ACT: evict PSUM -> bf16 SBUF with fused per-partition bias
  DMA out (bf16; host upcasts to f32).
Emission is software-pipelined: S1(x+3) is emitted before S5(x) so the PE
stream never blocks on the DVE y-conv of the current row.
"""

import os
import numpy as np
import ml_dtypes

import concourse.bass as bass
import concourse.bacc as bacc
import concourse.mybir as mybir
import concourse.tile as tile
from concourse.bass_utils import run_bass_kernel_spmd

BF16 = ml_dtypes.bfloat16

CIN, COUT, R, KK = 32, 64, 53, 3
H = W = D = 56
HP = 58            # padded spatial
NCORES = 8
XO = 7             # output H-rows per core
XR = 9             # input H-rows per core (with halo)
Z30 = 30           # z-window per z-half (28 out + 2 halo)
PLANE = HP * HP    # 3364
FD_X = XR * PLANE  # 30276
FD_T2 = HP * Z30   # 1740
FD_T3 = H * Z30    # 1680
FD_OX = H * 28     # 1568

_cache = {}


def _build_program():
    nc = bacc.Bacc("TRN2", debug=False, num_devices=NCORES)
    f32, bf16 = mybir.dt.float32, mybir.dt.bfloat16

    # pre-packed input: 3 H-shifted copies on partition blocks of 32
    x3_d = nc.dram_tensor("x3", [96, FD_X], mybir.dt.float8e4, kind="ExternalInput").ap()
    # packed weights: one bf16 tensor, one f32 tensor
    wb_d = nc.dram_tensor("wb", [128, 256], bf16, kind="ExternalInput").ap()
    wf_d = nc.dram_tensor("wf", [128, 4], f32, kind="ExternalInput").ap()
    # [zh, o, x, y, z28]: matches SBUF partition-major layout; host reassembles
    out_d = nc.dram_tensor("out", [2, COUT, XO, H, 28], bf16,
                           kind="ExternalOutput").ap()

    with tile.TileContext(nc) as tc:
        with (
            tc.tile_pool(name="const", bufs=1) as cpool,
            tc.tile_pool(name="work", bufs=3) as wpool,
            tc.tile_pool(name="outp", bufs=3) as opool,
            tc.tile_pool(name="ps1", bufs=2, space="PSUM") as ps1,
            tc.tile_pool(name="ps2", bufs=4, space="PSUM") as ps2,
        ):
            X3 = cpool.tile([96, FD_X], mybir.dt.float8e4)
            WB = cpool.tile([128, 256], bf16)   # w96 | wa0 | wa1 | wa2
            WF = cpool.tile([128, 4], f32)      # s0 s1 s2 biasT

            # weights via the ACT HWDGE ring; input chunks own the SP ring
            nc.scalar.dma_start(WB[:, :], wb_d[:, :])
            nc.scalar.dma_start(WF[:, :], wf_d[:, :])
            w96s = WB[0:96, 0:64]
            was = [WB[0:128, 64 * (l + 1):64 * (l + 2)] for l in range(KK)]
            ss = [WF[0:128, j:j + 1] for j in range(KK)]
            biasT = WF[0:128, 3:4]

            # chunked input DMAs (one per H-row plane); trim zero tails
            nc.sync.dma_start(X3[0:96, 0:29 * HP], x3_d[0:96, 0:29 * HP])
            nc.sync.dma_start(X3[0:96, 29 * HP:PLANE], x3_d[0:96, 29 * HP:PLANE])
            for r in range(1, XR):
                pmax = 96 if r < XO else (64 if r == XO else 32)
                nc.sync.dma_start(X3[0:pmax, r * PLANE:(r + 1) * PLANE],
                                  x3_d[0:pmax, r * PLANE:(r + 1) * PLANE])

            X3v = X3.rearrange("p (x y z) -> p x y z", x=XR, y=HP, z=HP)

            t2s, t3s = {}, {}

            def emit_s1(x):
                # S1: K=96 matmul + E1 eviction -> T2 bf16 [y58, z30]
                t2 = wpool.tile([128, FD_T2], bf16, name="t2", tag="t2", bufs=5)
                t2s[x] = t2
                for yh in range(2):
                    p1 = ps1.tile([128, 1024], f32, name="p1", tag="p1")
                    y0 = yh * 29
                    for zh in range(2):
                        for zc in range(2):   # z half-chunks of 15
                            nc.tensor.matmul(
                                p1[zh * 64:zh * 64 + 64,
                                   zc * 512:zc * 512 + 29 * 15],
                                w96s,
                                X3v[0:96, x, y0:y0 + 29,
                                    zh * 28 + zc * 15:zh * 28 + zc * 15 + 15],
                                start=True, stop=True,
                            )
                    # E1: one ACT op per psum tile (both zh blocks, both z-chunks)
                    src = p1.rearrange("p (zc w) -> p zc w", zc=2)[
                        0:128, 0:2, 0:29 * 15].rearrange(
                        "p zc (y z) -> p zc y z", y=29)
                    dst = t2.rearrange("p (y zc z) -> p zc y z", y=HP, zc=2)[
                        0:128, 0:2, y0:y0 + 29, 0:15]
                    nc.scalar.activation(
                        dst, src, mybir.ActivationFunctionType.Copy)

            def emit_yc(x):
                # y-conv: t3 = s0*t2[y] + s1*t2[y+1] + s2*t2[y+2]
                t2 = t2s.pop(x)
                t3 = wpool.tile([128, FD_T3], bf16, name="t3", tag="t3")
                t3b = wpool.tile([128, FD_T3], bf16, name="t3b", tag="t3b", bufs=2)
                t3c = wpool.tile([128, FD_T3], bf16, name="t3c", tag="t3c", bufs=2)
                t3s[x] = t3
                nc.vector.tensor_scalar_mul(t3[0:128, :], t2[0:128, 0:FD_T3], ss[0])
                nc.vector.tensor_scalar_mul(
                    t3b[0:128, :], t2[0:128, Z30:Z30 + FD_T3], ss[1])
                nc.vector.tensor_scalar_mul(
                    t3c[0:128, :], t2[0:128, 2 * Z30:2 * Z30 + FD_T3], ss[2])
                nc.vector.tensor_tensor(
                    t3[0:128, :], t3[0:128, :], t3b[0:128, :], mybir.AluOpType.add)
                nc.vector.tensor_tensor(
                    t3[0:128, :], t3[0:128, :], t3c[0:128, :], mybir.AluOpType.add)

            def emit_s5(x):
                # S5 + E2 + out-DMA; yc-pair psum tiles retire via one ACT op
                t3 = t3s.pop(x)
                t3v = t3.rearrange("p (y z) -> p y z", y=H, z=Z30)
                ox = opool.tile([128, FD_OX], mybir.dt.bfloat16, name="ox", tag="ox")
                for yc in range(4):
                    yb = yc * 14
                    p2 = ps2.tile([128, 512], f32, name="p2", tag="p2")
                    for l in range(KK):
                        for zh in range(2):
                            nc.tensor.matmul(
                                p2[zh * 64:zh * 64 + 64, 0:392],
                                was[l][zh * 64:zh * 64 + 53, 0:64],
                                t3v[zh * 64:zh * 64 + 53, yb:yb + 14, l:l + 28],
                                start=(l == 0), stop=(l == KK - 1),
                            )
                    if x % 2 == 1 and yc == 3:
                        # balance ACT vs DVE: biased eviction on DVE
                        nc.vector.tensor_scalar_add(
                            ox[0:128, yc * 392:yc * 392 + 392],
                            p2[0:128, 0:392], biasT)
                    else:
                        nc.scalar.activation(
                            ox[0:128, yc * 392:yc * 392 + 392],
                            p2[0:128, 0:392],
                            mybir.ActivationFunctionType.Identity,
                            bias=biasT,
                        )
                if x == XO - 1:
                    nc.sync.dma_start(out_d[0:2, 0:COUT, x, 0:28, 0:28],
                                      ox[0:128, 0:FD_OX // 2])
                    nc.sync.dma_start(out_d[0:2, 0:COUT, x, 28:H, 0:28],
                                      ox[0:128, FD_OX // 2:FD_OX])
                else:
                    nc.sync.dma_start(out_d[0:2, 0:COUT, x, 0:H, 0:28],
                                      ox[0:128, 0:FD_OX])

            # software-pipelined emission (3 rows ahead)
            emit_s1(0)
            emit_s1(1)
            emit_s1(2)
            for x in range(XO):
                emit_yc(x)
                if x + 3 < XO:
                    emit_s1(x + 3)
                emit_s5(x)

    nc.compile()
    return nc


def _prep_weights(U_kh, U_kw, U_kd, U_cin, U_cout, bias):
    U_kh, U_kw, U_kd = (np.asarray(a, np.float32) for a in (U_kh, U_kw, U_kd))
    U_cin, U_cout = np.asarray(U_cin, np.float32), np.asarray(U_cout, np.float32)
    bias = np.asarray(bias, np.float32)

    wb = np.zeros((128, 256), np.float32)
    wb[0:96, 0:64][:, :R] = (U_kh[:, None, :] * U_cin[None, :, :]).reshape(96, R)
    for l in range(KK):
        v = U_kd[l][:, None] * U_cout
        wb[0:R, 64 * (l + 1):64 * (l + 2)] = v
        wb[64:64 + R, 64 * (l + 1):64 * (l + 2)] = v

    wf = np.zeros((128, 4), np.float32)
    for j in range(KK):
        wf[0:R, j] = U_kw[j]
        wf[64:64 + R, j] = U_kw[j]
    wf[0:64, 3] = bias
    wf[64:128, 3] = bias
    return wb.astype(BF16), wf


def kernel(x, U_kh, U_kw, U_kd, U_cin, U_cout, bias):
    x = np.asarray(x, np.float32)
    assert x.shape == (1, CIN, H, W, D)

    if "nc" not in _cache:
        _cache["nc"] = _build_program()
    nc = _cache["nc"]

    wb, wf = _prep_weights(U_kh, U_kw, U_kd, U_cin, U_cout, bias)

    xp = np.zeros((CIN, HP, HP, HP), np.float32)
    xp[:, 1:57, 1:57, 1:57] = x[0]
    xp = xp.astype(ml_dtypes.float8_e4m3)

    in_maps = []
    for k in range(NCORES):
        shard = xp[:, 7 * k:7 * k + XR].reshape(CIN, FD_X)
        x3 = np.zeros((96, FD_X), ml_dtypes.float8_e4m3)
        x3[0:32] = shard
        x3[32:64, 0:FD_X - PLANE] = shard[:, PLANE:]
        x3[64:96, 0:FD_X - 2 * PLANE] = shard[:, 2 * PLANE:]
        in_maps.append({"x3": x3, "wb": wb, "wf": wf})

    trace = bool(int(os.environ.get("KERNEL_PROFILE", "0")))
    res = run_bass_kernel_spmd(nc, in_maps, core_ids=list(range(NCORES)),
                               trace=trace)
    if trace and res.exec_time_ns is not None:
        print(f"HW exec time: {res.exec_time_ns} ns")
        _cache["exec_time_ns"] = res.exec_time_ns

    out = np.empty((1, COUT, H, W, D), np.float32)
    for k in range(NCORES):
        r = np.asarray(res.results[k]["out"], np.float32).reshape(
            2, COUT, XO, H, 28)
        out[0, :, 7 * k:7 * k + XO] = r.transpose(1, 2, 3, 0, 4).reshape(
            COUT, XO, H, D)
    return out


if __name__ == "__main__":
    rng = np.random.default_rng(0)
    ins = {
        "x": rng.standard_normal((1, CIN, H, W, D)).astype(np.float32),
        "U_kh": (rng.standard_normal((KK, R)) * 0.1).astype(np.float32),
        "U_kw": (rng.standard_normal((KK, R)) * 0.1).astype(np.float32),
        "U_kd": (rng.standard_normal((KK, R)) * 0.1).astype(np.float32),
        "U_cin": (rng.standard_normal((CIN, R)) * 0.1).astype(np.float32),
        "U_cout": (rng.standard_normal((R, COUT)) * 0.1).astype(np.float32),
        "bias": rng.standard_normal((COUT,)).astype(np.float32),
    }
    o = kernel(**ins)
    print("kernel ran, out shape", o.shape, "mean", float(np.abs(o).mean()))


# revision 22
# speedup vs baseline: 1.1455x; 1.1455x over previous
"""CP-decomposed 3D conv (AirConv3D) on 8 Trainium2 NeuronCores.

Math (reference):
  out[o,X,Y,Z] = sum_{i,j,l,c,r} xp[c,X+i,Y+j,Z+l] * U_kh[i,r]*U_kw[j,r]*U_kd[l,r]
                  * U_cin[c,r]*U_cout[r,o] + bias[o]
  xp = zero-padded x (pad 1), kernel 3x3x3, CP rank 53.

Device pipeline per core (H-sharded, 7 output rows + 1-row halo each side):
  S1  TensorE: K=(i,c)=96 matmul folding the H-tap conv + channel contraction
      -> t2[r, Y58, Z30] per z-half (PSUM partition blocks 0/64 via col groups)
  E1  ACT: evict PSUM -> bf16 SBUF (T2)
  YC  DVE: 3-tap y-conv with per-partition scalars (3x tensor_scalar + 2x TT add)
  S5  TensorE: 3 accumulated z-shifted matmuls folding the D-tap conv and
      rank->cout contraction (K=53 per z-half; zh blocks run as concurrent
      row-group tiles)
  E2  ACT: evict PSUM -> bf16 SBUF with fused per-partition bias
  DMA out (bf16; host upcasts to f32).
Emission is software-pipelined: S1(x+3) is emitted before S5(x) so the PE
stream never blocks on the DVE y-conv of the current row.
"""

import os
import numpy as np
import ml_dtypes

import concourse.bass as bass
import concourse.bacc as bacc
import concourse.mybir as mybir
import concourse.tile as tile
from concourse.bass_utils import run_bass_kernel_spmd

BF16 = ml_dtypes.bfloat16

CIN, COUT, R, KK = 32, 64, 53, 3
H = W = D = 56
HP = 58            # padded spatial
NCORES = 8
XO = 7             # output H-rows per core
XR = 9             # input H-rows per core (with halo)
Z30 = 30           # z-window per z-half (28 out + 2 halo)
PLANE = HP * HP    # 3364
FD_X = XR * PLANE  # 30276
FD_T2 = HP * Z30   # 1740
FD_T3 = H * Z30    # 1680
FD_OX = H * 28     # 1568

_cache = {}


def _build_program():
    nc = bacc.Bacc("TRN2", debug=False, num_devices=NCORES)
    f32, bf16 = mybir.dt.float32, mybir.dt.bfloat16

    # pre-packed input: 3 H-shifted copies on partition blocks of 32
    x3_d = nc.dram_tensor("x3", [96, FD_X], mybir.dt.float8e4, kind="ExternalInput").ap()
    # packed weights: one bf16 tensor, one f32 tensor
    wb_d = nc.dram_tensor("wb", [128, 256], bf16, kind="ExternalInput").ap()
    wf_d = nc.dram_tensor("wf", [128, 4], f32, kind="ExternalInput").ap()
    # [zh, o, x, y, z28]: matches SBUF partition-major layout; host reassembles
    out_d = nc.dram_tensor("out", [2, COUT, XO, H, 28], bf16,
                           kind="ExternalOutput").ap()

    with tile.TileContext(nc) as tc:
        with (
            tc.tile_pool(name="const", bufs=1) as cpool,
            tc.tile_pool(name="work", bufs=3) as wpool,
            tc.tile_pool(name="outp", bufs=3) as opool,
            tc.tile_pool(name="ps1", bufs=2, space="PSUM") as ps1,
            tc.tile_pool(name="ps2", bufs=4, space="PSUM") as ps2,
        ):
            X3 = cpool.tile([96, FD_X], mybir.dt.float8e4)
            WB = cpool.tile([128, 256], bf16)   # w96 | wa0 | wa1 | wa2
            WF = cpool.tile([128, 4], f32)      # s0 s1 s2 biasT

            # weights via the ACT HWDGE ring; input chunks own the SP ring
            nc.scalar.dma_start(WB[:, :], wb_d[:, :])
            nc.scalar.dma_start(WF[:, :], wf_d[:, :])
            w96s = WB[0:96, 0:64]
            was = [WB[0:128, 64 * (l + 1):64 * (l + 2)] for l in range(KK)]
            ss = [WF[0:128, j:j + 1] for j in range(KK)]
            biasT = WF[0:128, 3:4]

            # chunked input DMAs (one per H-row plane); trim zero tails
            nc.sync.dma_start(X3[0:96, 0:29 * HP], x3_d[0:96, 0:29 * HP])
            nc.sync.dma_start(X3[0:96, 29 * HP:PLANE], x3_d[0:96, 29 * HP:PLANE])
            for r in range(1, XR):
                pmax = 96 if r < XO else (64 if r == XO else 32)
                nc.sync.dma_start(X3[0:pmax, r * PLANE:(r + 1) * PLANE],
                                  x3_d[0:pmax, r * PLANE:(r + 1) * PLANE])

            X3v = X3.rearrange("p (x y z) -> p x y z", x=XR, y=HP, z=HP)

            t2s, t3s = {}, {}

            def emit_s1(x):
                # S1: K=96 matmul + E1 eviction -> T2 bf16 [y58, z30]
                t2 = wpool.tile([128, FD_T2], bf16, name="t2", tag="t2", bufs=5)
                t2s[x] = t2
                for yh in range(2):
                    p1 = ps1.tile([128, 1024], f32, name="p1", tag="p1")
                    y0 = yh * 29
                    for zh in range(2):
                        for zc in range(2):   # z half-chunks of 15
                            nc.tensor.matmul(
                                p1[zh * 64:zh * 64 + 64,
                                   zc * 512:zc * 512 + 29 * 15],
                                w96s,
                                X3v[0:96, x, y0:y0 + 29,
                                    zh * 28 + zc * 15:zh * 28 + zc * 15 + 15],
                                start=True, stop=True,
                            )
                    # E1: one ACT op per psum tile (both zh blocks, both z-chunks)
                    src = p1.rearrange("p (zc w) -> p zc w", zc=2)[
                        0:128, 0:2, 0:29 * 15].rearrange(
                        "p zc (y z) -> p zc y z", y=29)
                    dst = t2.rearrange("p (y zc z) -> p zc y z", y=HP, zc=2)[
                        0:128, 0:2, y0:y0 + 29, 0:15]
                    nc.scalar.activation(
                        dst, src, mybir.ActivationFunctionType.Copy)

            def emit_yc(x):
                # y-conv: t3 = s0*t2[y] + s1*t2[y+1] + s2*t2[y+2]
                t2 = t2s.pop(x)
                t3 = wpool.tile([128, FD_T3], bf16, name="t3", tag="t3")
                t3b = wpool.tile([128, FD_T3], bf16, name="t3b", tag="t3b", bufs=2)
                t3c = wpool.tile([128, FD_T3], bf16, name="t3c", tag="t3c", bufs=2)
                t3s[x] = t3
                nc.vector.tensor_scalar_mul(t3[0:128, :], t2[0:128, 0:FD_T3], ss[0])
                nc.vector.tensor_scalar_mul(
                    t3b[0:128, :], t2[0:128, Z30:Z30 + FD_T3], ss[1])
                nc.vector.tensor_scalar_mul(
                    t3c[0:128, :], t2[0:128, 2 * Z30:2 * Z30 + FD_T3], ss[2])
                nc.vector.tensor_tensor(
                    t3[0:128, :], t3[0:128, :], t3b[0:128, :], mybir.AluOpType.add)
                nc.vector.tensor_tensor(
                    t3[0:128, :], t3[0:128, :], t3c[0:128, :], mybir.AluOpType.add)

            def emit_s5(x):
                # S5 + E2 + out-DMA; yc-pair psum tiles retire via one ACT op
                t3 = t3s.pop(x)
                t3v = t3.rearrange("p (y z) -> p y z", y=H, z=Z30)
                ox = opool.tile([128, FD_OX], mybir.dt.bfloat16, name="ox", tag="ox")
                for yc in range(4):
                    yb = yc * 14
                    p2 = ps2.tile([128, 512], f32, name="p2", tag="p2")
                    for l in range(KK):
                        for zh in range(2):
                            nc.tensor.matmul(
                                p2[zh * 64:zh * 64 + 64, 0:392],
                                was[l][zh * 64:zh * 64 + 53, 0:64],
                                t3v[zh * 64:zh * 64 + 53, yb:yb + 14, l:l + 28],
                                start=(l == 0), stop=(l == KK - 1),
                            )
                    nc.scalar.activation(
                        ox[0:128, yc * 392:yc * 392 + 392],
                        p2[0:128, 0:392],
                        mybir.ActivationFunctionType.Identity,
                        bias=biasT,
                    )
                if x == XO - 1:
                    nc.sync.dma_start(out_d[0:2, 0:COUT, x, 0:28, 0:28],
                                      ox[0:128, 0:FD_OX // 2])
                    nc.sync.dma_start(out_d[0:2, 0:COUT, x, 28:H, 0:28],
                                      ox[0:128, FD_OX // 2:FD_OX])
                else:
                    nc.sync.dma_start(out_d[0:2, 0:COUT, x, 0:H, 0:28],
                                      ox[0:128, 0:FD_OX])

            # software-pipelined emission (3 rows ahead)
            emit_s1(0)
            emit_s1(1)
            emit_s1(2)
            for x in range(XO):
                emit_yc(x)
                if x + 3 < XO:
                    emit_s1(x + 3)
                emit_s5(x)

    nc.compile()
    return nc


def _prep_weights(U_kh, U_kw, U_kd, U_cin, U_cout, bias):
    U_kh, U_kw, U_kd = (np.asarray(a, np.float32) for a in (U_kh, U_kw, U_kd))
    U_cin, U_cout = np.asarray(U_cin, np.float32), np.asarray(U_cout, np.float32)
    bias = np.asarray(bias, np.float32)

    wb = np.zeros((128, 256), np.float32)
    wb[0:96, 0:64][:, :R] = (U_kh[:, None, :] * U_cin[None, :, :]).reshape(96, R)
    for l in range(KK):
        v = U_kd[l][:, None] * U_cout
        wb[0:R, 64 * (l + 1):64 * (l + 2)] = v
        wb[64:64 + R, 64 * (l + 1):64 * (l + 2)] = v

    wf = np.zeros((128, 4), np.float32)
    for j in range(KK):
        wf[0:R, j] = U_kw[j]
        wf[64:64 + R, j] = U_kw[j]
    wf[0:64, 3] = bias
    wf[64:128, 3] = bias
    return wb.astype(BF16), wf


def kernel(x, U_kh, U_kw, U_kd, U_cin, U_cout, bias):
    x = np.asarray(x, np.float32)
    assert x.shape == (1, CIN, H, W, D)

    if "nc" not in _cache:
        _cache["nc"] = _build_program()
    nc = _cache["nc"]

    wb, wf = _prep_weights(U_kh, U_kw, U_kd, U_cin, U_cout, bias)

    xp = np.zeros((CIN, HP, HP, HP), np.float32)
    xp[:, 1:57, 1:57, 1:57] = x[0]
    xp = xp.astype(ml_dtypes.float8_e4m3)

    in_maps = []
    for k in range(NCORES):
        shard = xp[:, 7 * k:7 * k + XR].reshape(CIN, FD_X)
        x3 = np.zeros((96, FD_X), ml_dtypes.float8_e4m3)
        x3[0:32] = shard
        x3[32:64, 0:FD_X - PLANE] = shard[:, PLANE:]
        x3[64:96, 0:FD_X - 2 * PLANE] = shard[:, 2 * PLANE:]
        in_maps.append({"x3": x3, "wb": wb, "wf": wf})

    trace = bool(int(os.environ.get("KERNEL_PROFILE", "0")))
    res = run_bass_kernel_spmd(nc, in_maps, core_ids=list(range(NCORES)),
                               trace=trace)
    if trace and res.exec_time_ns is not None:
        print(f"HW exec time: {res.exec_time_ns} ns")
        _cache["exec_time_ns"] = res.exec_time_ns

    out = np.empty((1, COUT, H, W, D), np.float32)
    for k in range(NCORES):
        r = np.asarray(res.results[k]["out"], np.float32).reshape(
            2, COUT, XO, H, 28)
        out[0, :, 7 * k:7 * k + XO] = r.transpose(1, 2, 3, 0, 4).reshape(
            COUT, XO, H, D)
    return out


if __name__ == "__main__":
    rng = np.random.default_rng(0)
    ins = {
        "x": rng.standard_normal((1, CIN, H, W, D)).astype(np.float32),
        "U_kh": (rng.standard_normal((KK, R)) * 0.1).astype(np.float32),
        "U_kw": (rng.standard_normal((KK, R)) * 0.1).astype(np.float32),
        "U_kd": (rng.standard_normal((KK, R)) * 0.1).astype(np.float32),
        "U_cin": (rng.standard_normal((CIN, R)) * 0.1).astype(np.float32),
        "U_cout": (rng.standard_normal((R, COUT)) * 0.1).astype(np.float32),
        "bias": rng.standard_normal((COUT,)).astype(np.float32),
    }
    o = kernel(**ins)
    print("kernel ran, out shape", o.shape, "mean", float(np.abs(o).mean()))
